# revision 34
# baseline (speedup 1.0000x reference)
"""DynamicA8W8 MoE FFN on 8 TRN2 NeuronCores.

Sizes (hardcoded from the problem spec):
  T=4096 tokens, H=4096 hidden, I=1408 intermediate, E=16 experts,
  equal contiguous groups of TPE=256 tokens per expert.

Sharding: expert-parallel == token-parallel here (contiguous equal groups).
Core c owns experts {2c, 2c+1} and tokens [512c, 512c+512). No cross-core
communication is needed; each core computes its own [512, H] output slab and
the host concatenates.

Per-core pipeline:
  1. per-token dynamic quant of x -> int8 (RNE+saturate via f32->int8 copy),
     exact in bf16; PE-transpose to [h, t] layout for use as matmul stationary.
  2. grouped GEMM1 vs w13 (int8 weights DMA'd raw, cast to bf16 on chip;
     bf16 matmul is exact for int8 operands, fp32 PSUM accumulate).
  3. SwiGLU epilogue fused with dequant scales, dynamic requant to int8.
  4. GEMM2 vs w2, fused per-channel + per-token dequant, DMA out.
"""

import json

import numpy as np

import concourse.bass as bass
import concourse.bass2jax as bass2jax
import concourse.mybir as mybir
from concourse.bass_utils import run_bass_kernel_spmd
from concourse.masks import make_identity
from concourse.tile import TileContext

F32 = mybir.dt.float32
BF16 = mybir.dt.bfloat16
I8 = mybir.dt.int8
AF = mybir.ActivationFunctionType
ALU = mybir.AluOpType
AX = mybir.AxisListType

T, H, I, E = 4096, 4096, 1408, 16
NCORES = 8
E_LOC = E // NCORES            # 2 experts per core
TPE = T // E                   # 256 tokens per expert
T_LOC = E_LOC * TPE            # 512 tokens per core
NTB = T_LOC // 128             # 4 token blocks per core
KT1 = H // 128                 # 32 k-tiles for mm1
KT2 = I // 128                 # 11 k-tiles for mm2
# gate/up column chunks (free dim of mm1, <=512 per PSUM bank)
I_CHUNKS = [(0, 512), (512, 512), (1024, 384)]
H_CHUNKS = [(c, 512) for c in range(0, H, 512)]


# --- walrus workaround: this build rejects >1 sync wait per instruction.
# Split extras into standalone single-wait EventSemaphore instructions placed
# immediately before, on the same engine queue.
def _split_multi_waits(bir_json: bytes) -> bytes:
    j = json.loads(bir_json)
    changed = False
    for fn in j.get("functions", []):
        for blk in fn.get("blocks", []):
            out = []
            for inst in blk.get("instructions", []):
                si = inst.get("sync_info")
                waits = si.get("on_wait") if si else None
                if waits and len(waits) > 1:
                    spill, keep = waits[:-1], waits[-1:]
                    for k, w in enumerate(spill):
                        out.append({
                            "debug": inst.get("debug", 0),
                            "engine": inst["engine"],
                            "ins": [], "outs": [],
                            "name": f"{inst['name']}_w{k}",
                            "opcode": "EventSemaphore",
                            "sync_info": {"on_update": [], "on_wait": [w]},
                        })
                    si["on_wait"] = keep
                    changed = True
                out.append(inst)
            blk["instructions"] = out
    return json.dumps(j).encode() if changed else bir_json


_hook_installed = False


def _install_compile_hook():
    global _hook_installed
    if _hook_installed:
        return
    orig = bass2jax.compile_bir_kernel

    def wrapped(bir_json, tmpdir, neff_name="file.neff"):
        return orig(_split_multi_waits(bir_json), tmpdir, neff_name=neff_name)

    bass2jax.compile_bir_kernel = wrapped
    _hook_installed = True


def _cast_engine(nc, idx):
    """Round-robin the int8->bf16 weight casts across ACT/Pool/DVE.

    Balance for the engine rates (ACT 1.2G, DVE 0.96G, Pool ~0.72G effective)
    and each engine's other work: ACT 3/8, Pool 3/8, DVE 2/8.
    """
    # HW-measured int8->bf16 rates (ns per lane-elem): ACT 0.86, DVE 0.78,
    # Pool 3.9 (gpsimd is ~4x slower than the cost model thinks, and one slow
    # cast on the critical path stalls 8 matmuls) -- so no Pool casts at all.
    r = idx % 9
    if r < 5:
        return nc.scalar.copy
    return nc.vector.tensor_copy


def _build_program(reps=1):
    nc = bass.Bass()
    x_d = nc.declare_dram_parameter("x", [T_LOC, H], F32, isOutput=False)
    xT_d = nc.declare_dram_parameter("xT", [H, T_LOC], F32, isOutput=False)
    w13T_d = nc.declare_dram_parameter("w13T", [E_LOC, H, 2 * I], I8, isOutput=False)
    w2T_d = nc.declare_dram_parameter("w2T", [E_LOC, I, H], I8, isOutput=False)
    wsg_d = nc.declare_dram_parameter("wsg", [E_LOC, 128, I], F32, isOutput=False)
    wsu_d = nc.declare_dram_parameter("wsu", [E_LOC, 128, I], F32, isOutput=False)
    w2s_d = nc.declare_dram_parameter("w2s", [E_LOC, 128, H], F32, isOutput=False)
    out_d = nc.declare_dram_parameter("out", [T_LOC, H], F32, isOutput=True)

    with TileContext(nc) as tc:
        with (
            tc.tile_pool(name="const", bufs=1) as const,
            tc.tile_pool(name="xload", bufs=4) as xload,
            tc.tile_pool(name="xq", bufs=1) as xqp,
            tc.tile_pool(name="xqt", bufs=3) as xqtp,
            tc.tile_pool(name="small", bufs=4) as small,
            tc.tile_pool(name="wload", bufs=2) as wload,
            tc.tile_pool(name="wcast", bufs=6) as wcast,
            tc.tile_pool(name="scales", bufs=2) as scalep,
            tc.tile_pool(name="w2scale", bufs=1) as w2scalep,
            tc.tile_pool(name="hbuf", bufs=2) as hbuf,
            tc.tile_pool(name="hq", bufs=2) as hqp,
            tc.tile_pool(name="outp", bufs=2) as outp,
            tc.tile_pool(name="pt", bufs=2, space="PSUM") as ptp,
            tc.tile_pool(name="pg", bufs=2, space="PSUM") as pgp,
            tc.tile_pool(name="pu", bufs=2, space="PSUM") as pup,
            tc.tile_pool(name="p2", bufs=2, space="PSUM") as p2p,
        ):
            env = dict(locals())
            ident = const.tile([128, 128], BF16)
            make_identity(nc, ident)
            env["ident"] = ident
            ident_f32 = const.tile([128, 128], F32)
            make_identity(nc, ident_f32)
            env["ident_f32"] = ident_f32
            ones_row = const.tile([128, 128], F32)
            nc.vector.memset(ones_row[:], 1.0)
            env["ones_row"] = ones_row
            for _rep in range(reps):
                if _rep > 0:
                    env["out_d"] = nc.dram_tensor(
                        f"out_rep{_rep}", [T_LOC, H], F32).ap()
                _emit_body(nc, tc, env)
    return nc


def _emit_body(nc, tc, pools):
    const = pools["const"]; xload = pools["xload"]; xqp = pools["xq"] if "xq" in pools else pools["xqp"]
    xqp = pools["xqp"]; xqtp = pools["xqtp"]; small = pools["small"]
    wload = pools["wload"]; wcast = pools["wcast"]; scalep = pools["scalep"]
    w2scalep = pools["w2scalep"]; hbuf = pools["hbuf"]; hqp = pools["hqp"]
    outp = pools["outp"]; ptp = pools["ptp"]; pgp = pools["pgp"]
    pup = pools["pup"]; p2p = pools["p2p"]
    x_d = pools["x_d"]; w13T_d = pools["w13T_d"]; w2T_d = pools["w2T_d"]
    xT_d = pools["xT_d"]
    wsg_d = pools["wsg_d"]; wsu_d = pools["wsu_d"]; w2s_d = pools["w2s_d"]
    out_d = pools["out_d"]
    ident = pools["ident"]
    ident_f32 = pools["ident_f32"]
    ones_row = pools["ones_row"]

    xqT = {}     # t-block -> [128h, KT1, 128t] bf16
    s1s = {}     # t-block -> [128, 1] f32 quant scale
    cast_n = [0]

    def cast(dst, src):
        _cast_engine(nc, cast_n[0])(dst, src)
        cast_n[0] += 1

    def quantize_tb(tb):
        # amax over the natural-layout row quarters (free-dim reduce)...
        NQ = 4
        QW = H // NQ
        am = None
        for hh in range(NQ):
            xt = xload.tile([128, QW], F32, tag="xt", name=f"xt{tb}_{hh}",
                            bufs=2)
            nc.sync.dma_start(
                xt[:], x_d[tb * 128:(tb + 1) * 128, hh * QW:(hh + 1) * QW])
            amn = small.tile([128, 1], F32, tag="amax1", name=f"am{tb}_{hh}")
            nc.vector.tensor_reduce(amn[:], xt[:], axis=AX.X, op=ALU.max,
                                    apply_absolute_value=True)
            if hh > 0:
                am2 = small.tile([128, 1], F32, tag="amax1b",
                                 name=f"amc{tb}_{hh}")
                nc.vector.tensor_tensor(am2[:], am[:], amn[:], op=ALU.max)
                am = am2
            else:
                am = amn
        s1 = small.tile([128, 1], F32, tag="s1")
        nc.vector.tensor_scalar(s1[:], am[:], 1.0 / 127.0, None, op0=ALU.mult)
        inv1 = small.tile([128, 1], F32, tag="inv1")
        nc.vector.reciprocal(inv1[:], s1[:])
        # ...then quantize the host-pretransposed xT directly in [h, t]
        # layout: broadcast inv1 across partitions with a PE outer product
        # (ones (x) inv1^T), multiply + round to int8, cast to bf16. No
        # 128x128 data transposes through the PE at all.
        pinv = ptp.tile([128, 128], F32, tag="pt", name="pinv")
        nc.tensor.transpose(pinv[0:1, :], inv1[:], ident_f32[:])
        invrow = small.tile([128, 128], F32, tag="invrow")
        nc.vector.tensor_copy(invrow[0:1, :], pinv[0:1, :])
        pbc = ptp.tile([128, 128], F32, tag="pt", name="pbc")
        nc.tensor.matmul(pbc[:], ones_row[0:1, :], invrow[0:1, :],
                         start=True, stop=True)
        invb = small.tile([128, 1, 128], F32, tag="invb")
        nc.vector.tensor_copy(invb[:, 0, :], pbc[:])
        xqt = xqtp.tile([128, KT1, 128], BF16, tag="xqT")
        KQ = KT1 // NQ
        for hh in range(NQ):
            xTt = xload.tile([128, KQ, 128], F32, tag="xTt",
                             name=f"xTt{tb}_{hh}", bufs=2)
            nc.sync.dma_start(
                xTt[:],
                xT_d[hh * QW:(hh + 1) * QW, tb * 128:(tb + 1) * 128]
                .rearrange("(k p) t -> p k t", p=128))
            xq8 = xqp.tile([128, KQ, 128], I8, tag="xq_i8",
                           name=f"xq8_{hh}", bufs=2)
            nc.vector.scalar_tensor_tensor(
                xq8[:], xTt[:], 1.0, invb[:].broadcast_to([128, KQ, 128]),
                op0=ALU.mult, op1=ALU.mult)
            nc.scalar.copy(xqt[:, hh * KQ:(hh + 1) * KQ, :], xq8[:])
        xqT[tb] = xqt
        s1s[tb] = s1

    def mm1_loads(e, c0, cw):
        wg_i8 = [wload.tile([128, KT1 // 2, cw], I8, tag="wg_i8",
                            name=f"wg_i8_{e}_{c0}_{h2}") for h2 in range(2)]
        wu_i8 = [wload.tile([128, KT1 // 2, cw], I8, tag="wu_i8",
                            name=f"wu_i8_{e}_{c0}_{h2}") for h2 in range(2)]
        g_src = w13T_d[e, :, c0:c0 + cw].rearrange("(k p) o -> p k o", p=128)
        u_src = w13T_d[e, :, I + c0:I + c0 + cw].rearrange(
            "(k p) o -> p k o", p=128)
        for h2 in range(2):
            ksl = slice(h2 * (KT1 // 2), (h2 + 1) * (KT1 // 2))
            nc.sync.dma_start(wg_i8[h2][:], g_src[:, ksl, :])
            nc.sync.dma_start(wu_i8[h2][:], u_src[:, ksl, :])
        return wg_i8, wu_i8

    QK = 4  # k-tiles per cast op

    def cast_quad(w_i8, kq, cw, nm):
        h2, kkq = divmod(kq, (KT1 // 2) // QK)
        ks = slice(kkq * QK, (kkq + 1) * QK)
        w_bf = wcast.tile([128, QK, cw], BF16, tag="wbf", name=nm)
        cast(w_bf[:], w_i8[h2][:, ks, :])
        return w_bf

    def mm1_epilogue(e, i_tb, tb, c0, cw, pg, pu, wsg, wsu, hts, amaxes):
        gate = outp.tile([128, cw], F32, tag="gate")
        nc.vector.scalar_tensor_tensor(
            gate[:], pg, s1s[tb][:], wsg[:, c0:c0 + cw],
            op0=ALU.mult, op1=ALU.mult)
        up = outp.tile([128, cw], F32, tag="up")
        nc.vector.scalar_tensor_tensor(
            up[:], pu, s1s[tb][:], wsu[:, c0:c0 + cw],
            op0=ALU.mult, op1=ALU.mult)
        sg = outp.tile([128, cw], F32, tag="sg")
        nc.scalar.activation(sg[:], gate[:], AF.Silu)
        nc.vector.tensor_mul(hts[i_tb][:, c0:c0 + cw], sg[:], up[:])
        # per-chunk partial abs-max keeps the requant scale off the
        # critical path (ready right after the last chunk's h lands)
        prev = amaxes[i_tb]
        amp = small.tile([128, 1], F32, tag="amax2", name=f"am2p_{i_tb}_{c0}")
        nc.vector.tensor_reduce(amp[:], hts[i_tb][:, c0:c0 + cw],
                                axis=AX.X, op=ALU.max,
                                apply_absolute_value=True)
        if prev is not None:
            amn = small.tile([128, 1], F32, tag="amax2",
                             name=f"am2_{i_tb}_{c0}")
            nc.vector.tensor_tensor(amn[:], prev[:], amp[:], op=ALU.max)
            amaxes[i_tb] = amn
        else:
            amaxes[i_tb] = amp

    def mm1_chunk(e, tbs, c0, cw, wsg, wsu, hts, amaxes, loads,
                  precast=None):
        wg_i8, wu_i8 = loads
        pg = [pgp.tile([128, cw], F32, tag="pg", name=f"pg{i}")
              for i in range(len(tbs))]
        pu = [pup.tile([128, cw], F32, tag="pu", name=f"pu{i}")
              for i in range(len(tbs))]
        for kq in range(KT1 // QK):
            if precast is not None and kq < len(precast):
                wg_bf, wu_bf = precast[kq]
            else:
                wg_bf = cast_quad(wg_i8, kq, cw, "wg_bf")
                wu_bf = cast_quad(wu_i8, kq, cw, "wu_bf")
            for dk in range(QK):
                k = kq * QK + dk
                st, sp = (k == 0), (k == KT1 - 1)
                for i_tb, tb in enumerate(tbs):
                    nc.tensor.matmul(pg[i_tb][:], xqT[tb][:, k, :],
                                     wg_bf[:, dk, :], start=st, stop=sp)
                    nc.tensor.matmul(pu[i_tb][:], xqT[tb][:, k, :],
                                     wu_bf[:, dk, :], start=st, stop=sp)
        for i_tb, tb in enumerate(tbs):
            mm1_epilogue(e, i_tb, tb, c0, cw, pg[i_tb][:], pu[i_tb][:],
                         wsg, wsu, hts, amaxes)

    def mm1_chunk_split(e, tbs, c0, cw, wsg, wsu, hts, amaxes, loads, hqT,
                        s2s):
        # last chunk: separate per-tb passes (own casts) so tb0's requant
        # chain runs under tb1's matmuls instead of stalling the PE
        wg_i8, wu_i8 = loads
        for i_tb, tb in enumerate(tbs):
            pg = pgp.tile([128, cw], F32, tag="pg", name=f"pgs{i_tb}")
            pu = pup.tile([128, cw], F32, tag="pu", name=f"pus{i_tb}")
            for kq in range(KT1 // QK):
                wg_bf = cast_quad(wg_i8, kq, cw, "wg_bf")
                wu_bf = cast_quad(wu_i8, kq, cw, "wu_bf")
                for dk in range(QK):
                    k = kq * QK + dk
                    st, sp = (k == 0), (k == KT1 - 1)
                    nc.tensor.matmul(pg[:], xqT[tb][:, k, :],
                                     wg_bf[:, dk, :], start=st, stop=sp)
                    nc.tensor.matmul(pu[:], xqT[tb][:, k, :],
                                     wu_bf[:, dk, :], start=st, stop=sp)
            mm1_epilogue(e, i_tb, tb, c0, cw, pg[:], pu[:], wsg, wsu, hts,
                         amaxes)
            hqt, s2 = requant_tb(hts[i_tb], amaxes[i_tb])
            hqT.append(hqt)
            s2s.append(s2)

    def requant_tb(ht, amax2):
        s2 = small.tile([128, 1], F32, tag="s2")
        nc.vector.tensor_scalar(s2[:], amax2[:], 1.0 / 127.0, None,
                                op0=ALU.mult)
        inv2 = small.tile([128, 1], F32, tag="inv2")
        nc.vector.reciprocal(inv2[:], s2[:])
        hq_i8 = hqp.tile([128, I], I8, tag="hq_i8")
        nc.vector.tensor_scalar(hq_i8[:], ht[:], inv2[:], None, op0=ALU.mult)
        hq_bf = hqp.tile([128, I], BF16, tag="hq_bf")
        nc.scalar.copy(hq_bf[:], hq_i8[:])
        hqt = hqp.tile([128, KT2, 128], BF16, tag="hqT")
        for k0 in range(0, KT2, 8):
            kn = min(8, KT2 - k0)
            pt = ptp.tile([128, 8, 128], BF16, tag="pt", name="pt_hq")
            for dk in range(kn):
                k = k0 + dk
                nc.tensor.transpose(pt[:, dk, :],
                                    hq_bf[:, k * 128:(k + 1) * 128], ident[:])
            (nc.scalar.copy if (k0 // 8) % 2 else nc.vector.tensor_copy)(
                hqt[:, k0:k0 + kn, :], pt[:, 0:kn, :])
        return hqt, s2

    def mm2_chunk(e, tbs, c0, cw, hqT, s2s, w2s):
        w2_i8 = wload.tile([128, KT2, cw], I8, tag="w2_i8")
        nc.sync.dma_start(
            w2_i8[:],
            w2T_d[e, :, c0:c0 + cw].rearrange("(k p) o -> p k o", p=128))
        p2 = [p2p.tile([128, cw], F32, tag="p2", name=f"p2_{i}")
              for i in range(len(tbs))]
        k = 0
        for qn in (4, 4, 3):
            w2_bf = wcast.tile([128, qn, cw], BF16, tag="wbf", name="w2_bf")
            cast(w2_bf[:], w2_i8[:, k:k + qn, :])
            for dk in range(qn):
                for i_tb in range(2):
                    nc.tensor.matmul(p2[i_tb][:], hqT[i_tb][:, k, :],
                                     w2_bf[:, dk, :], start=(k == 0),
                                     stop=(k == KT2 - 1))
                k += 1
        for i_tb, tb in enumerate(tbs):
            ot = outp.tile([128, cw], F32, tag="ot", bufs=4)
            nc.vector.scalar_tensor_tensor(
                ot[:], p2[i_tb][:], s2s[i_tb][:], w2s[:, c0:c0 + cw],
                op0=ALU.mult, op1=ALU.mult)
            nc.sync.dma_start(out_d[tb * 128:(tb + 1) * 128, c0:c0 + cw],
                              ot[:])

    # ---- Expert loop ----
    for e in range(E_LOC):
        tbs = [2 * e, 2 * e + 1]
        # chunk-0 weight DMAs + first quad casts go out before the quant
        # chain so the first matmuls have their moving operands ready
        loads0 = mm1_loads(e, *I_CHUNKS[0])
        precast = []
        if e == 0:
            for kq in range(2):
                precast.append((
                    cast_quad(loads0[0], kq, I_CHUNKS[0][1], "wg_bf"),
                    cast_quad(loads0[1], kq, I_CHUNKS[0][1], "wu_bf")))
        for tb in tbs:
            quantize_tb(tb)

        wsg = scalep.tile([128, I], F32, tag="wsg")
        nc.sync.dma_start(wsg[:], wsg_d[e])
        wsu = scalep.tile([128, I], F32, tag="wsu")
        nc.sync.dma_start(wsu[:], wsu_d[e])

        hts = [hbuf.tile([128, I], F32, tag="ht", name=f"ht{e}_{i}")
               for i in range(len(tbs))]
        amaxes = [None, None]
        hqT = []
        s2s = []
        loads = loads0
        for ci, (c0, cw) in enumerate(I_CHUNKS):
            if loads is None:
                loads = mm1_loads(e, c0, cw)
            mm1_chunk(e, tbs, c0, cw, wsg, wsu, hts, amaxes, loads,
                      precast=precast if ci == 0 else None)
            loads = None

        for i_tb in range(len(tbs)):
            hqt, s2 = requant_tb(hts[i_tb], amaxes[i_tb])
            hqT.append(hqt)
            s2s.append(s2)

        w2s = w2scalep.tile([128, H], F32, tag="w2s")
        nc.sync.dma_start(w2s[:], w2s_d[e])
        for (c0, cw) in H_CHUNKS:
            mm2_chunk(e, tbs, c0, cw, hqT, s2s, w2s)


_cached_nc = None


def _make_in_maps(x, w13, w2, w13_scale, smooth_scale_2, w2_scale):
    x = np.asarray(x, dtype=np.float32)
    w13 = np.asarray(w13).astype(np.int8, copy=False)
    w2 = np.asarray(w2).astype(np.int8, copy=False)
    w13_scale = np.asarray(w13_scale, dtype=np.float32)
    smooth_scale_2 = np.asarray(smooth_scale_2, dtype=np.float32)
    w2_scale = np.asarray(w2_scale, dtype=np.float32)

    # Fold the (linear) smooth scale into the up-projection dequant scale.
    wsu_full = w13_scale[:, I:] * smooth_scale_2          # [E, I]
    wsg_full = w13_scale[:, :I]                           # [E, I]

    in_maps = []
    for c in range(NCORES):
        es = slice(E_LOC * c, E_LOC * (c + 1))
        ts = slice(T_LOC * c, T_LOC * (c + 1))
        in_maps.append({
            "x": np.ascontiguousarray(x[ts]),
            "xT": np.ascontiguousarray(x[ts].T),
            "w13T": np.ascontiguousarray(w13[es].transpose(0, 2, 1)),
            "w2T": np.ascontiguousarray(w2[es].transpose(0, 2, 1)),
            "wsg": np.ascontiguousarray(
                np.broadcast_to(wsg_full[es, None, :], (E_LOC, 128, I))),
            "wsu": np.ascontiguousarray(
                np.broadcast_to(wsu_full[es, None, :], (E_LOC, 128, I))),
            "w2s": np.ascontiguousarray(
                np.broadcast_to(w2_scale[es, None, :], (E_LOC, 128, H))),
        })
    return in_maps


def _run(in_maps, **kwargs):
    global _cached_nc
    _install_compile_hook()
    if _cached_nc is None:
        _cached_nc = _build_program()
    return run_bass_kernel_spmd(_cached_nc, in_maps, list(range(NCORES)),
                                **kwargs)


def kernel(x, w13, w2, w13_scale, smooth_scale_2, w2_scale, expert_tokens):
    # expert_tokens describes the fixed equal contiguous grouping (the
    # reference ignores it); we rely on that same grouping.
    del expert_tokens
    in_maps = _make_in_maps(x, w13, w2, w13_scale, smooth_scale_2, w2_scale)
    res = _run(in_maps)
    return np.concatenate([res.results[c]["out"] for c in range(NCORES)],
                          axis=0)


def run_profiled(x, w13, w2, w13_scale, smooth_scale_2, w2_scale,
                 expert_tokens):
    """test.py helper: run with NTFF profiling, return BassKernelResults."""
    del expert_tokens
    in_maps = _make_in_maps(x, w13, w2, w13_scale, smooth_scale_2, w2_scale)
    return _run(in_maps, trace=True)


# revision 37
# speedup vs baseline: 1.8401x; 1.8401x over previous
"""DynamicA8W8 MoE FFN on 8 TRN2 NeuronCores.

Sizes (hardcoded from the problem spec):
  T=4096 tokens, H=4096 hidden, I=1408 intermediate, E=16 experts,
  equal contiguous groups of TPE=256 tokens per expert.

Sharding: expert-parallel == token-parallel here (contiguous equal groups).
Core c owns experts {2c, 2c+1} and tokens [512c, 512c+512). No cross-core
communication is needed; each core computes its own [512, H] output slab and
the host concatenates.

Per-core pipeline:
  1. per-token dynamic quant of x -> int8 (RNE+saturate via f32->int8 copy),
     exact in bf16; PE-transpose to [h, t] layout for use as matmul stationary.
  2. grouped GEMM1 vs w13 (int8 weights DMA'd raw, cast to bf16 on chip;
     bf16 matmul is exact for int8 operands, fp32 PSUM accumulate).
  3. SwiGLU epilogue fused with dequant scales, dynamic requant to int8.
  4. GEMM2 vs w2, fused per-channel + per-token dequant, DMA out.
"""

import json

import numpy as np

import concourse.bass as bass
import concourse.bass2jax as bass2jax
import concourse.mybir as mybir
from concourse.bass_utils import run_bass_kernel_spmd
from concourse.masks import make_identity
from concourse.tile import TileContext

F32 = mybir.dt.float32
BF16 = mybir.dt.bfloat16
I8 = mybir.dt.int8
AF = mybir.ActivationFunctionType
ALU = mybir.AluOpType
AX = mybir.AxisListType

T, H, I, E = 4096, 4096, 1408, 16
NCORES = 8
E_LOC = E // NCORES            # 2 experts per core
TPE = T // E                   # 256 tokens per expert
T_LOC = E_LOC * TPE            # 512 tokens per core
NTB = T_LOC // 128             # 4 token blocks per core
KT1 = H // 128                 # 32 k-tiles for mm1
KT2 = I // 128                 # 11 k-tiles for mm2
# gate/up column chunks (free dim of mm1, <=512 per PSUM bank)
I_CHUNKS = [(0, 512), (512, 512), (1024, 384)]
H_CHUNKS = [(c, 512) for c in range(0, H, 512)]


# --- walrus workaround: this build rejects >1 sync wait per instruction.
# Split extras into standalone single-wait EventSemaphore instructions placed
# immediately before, on the same engine queue.
def _split_multi_waits(bir_json: bytes) -> bytes:
    j = json.loads(bir_json)
    changed = False
    for fn in j.get("functions", []):
        for blk in fn.get("blocks", []):
            out = []
            for inst in blk.get("instructions", []):
                si = inst.get("sync_info")
                waits = si.get("on_wait") if si else None
                if waits and len(waits) > 1:
                    spill, keep = waits[:-1], waits[-1:]
                    for k, w in enumerate(spill):
                        out.append({
                            "debug": inst.get("debug", 0),
                            "engine": inst["engine"],
                            "ins": [], "outs": [],
                            "name": f"{inst['name']}_w{k}",
                            "opcode": "EventSemaphore",
                            "sync_info": {"on_update": [], "on_wait": [w]},
                        })
                    si["on_wait"] = keep
                    changed = True
                out.append(inst)
            blk["instructions"] = out
    return json.dumps(j).encode() if changed else bir_json


_hook_installed = False


def _install_compile_hook():
    global _hook_installed
    if _hook_installed:
        return
    orig = bass2jax.compile_bir_kernel

    def wrapped(bir_json, tmpdir, neff_name="file.neff"):
        return orig(_split_multi_waits(bir_json), tmpdir, neff_name=neff_name)

    bass2jax.compile_bir_kernel = wrapped
    _hook_installed = True


def _cast_engine(nc, idx):
    """Round-robin the int8->bf16 weight casts across ACT/Pool/DVE.

    Balance for the engine rates (ACT 1.2G, DVE 0.96G, Pool ~0.72G effective)
    and each engine's other work: ACT 3/8, Pool 3/8, DVE 2/8.
    """
    # HW-measured int8->bf16 rates (ns per lane-elem): ACT 0.86, DVE 0.78,
    # Pool 3.9 (gpsimd is ~4x slower than the cost model thinks, and one slow
    # cast on the critical path stalls 8 matmuls) -- so no Pool casts at all.
    r = idx % 9
    if r < 5:
        return nc.scalar.copy
    return nc.vector.tensor_copy


def _build_program(reps=1):
    nc = bass.Bass()
    x_d = nc.declare_dram_parameter("x", [T_LOC, H], F32, isOutput=False)
    xT_d = nc.declare_dram_parameter("xT", [H, T_LOC], F32, isOutput=False)
    w13T_d = nc.declare_dram_parameter("w13T", [E_LOC, H, 2 * I], I8, isOutput=False)
    w2T_d = nc.declare_dram_parameter("w2T", [E_LOC, I, H], I8, isOutput=False)
    wsg_d = nc.declare_dram_parameter("wsg", [E_LOC, 128, I], F32, isOutput=False)
    wsu_d = nc.declare_dram_parameter("wsu", [E_LOC, 128, I], F32, isOutput=False)
    w2s_d = nc.declare_dram_parameter("w2s", [E_LOC, 128, H], F32, isOutput=False)
    out_d = nc.declare_dram_parameter("out", [T_LOC, H], F32, isOutput=True)

    with TileContext(nc) as tc:
        with (
            tc.tile_pool(name="const", bufs=1) as const,
            tc.tile_pool(name="xload", bufs=4) as xload,
            tc.tile_pool(name="xq", bufs=1) as xqp,
            tc.tile_pool(name="xqt", bufs=4) as xqtp,
            tc.tile_pool(name="small", bufs=4) as small,
            tc.tile_pool(name="wload", bufs=2) as wload,
            tc.tile_pool(name="wcast", bufs=6) as wcast,
            tc.tile_pool(name="scales", bufs=2) as scalep,
            tc.tile_pool(name="w2scale", bufs=1) as w2scalep,
            tc.tile_pool(name="hbuf", bufs=2) as hbuf,
            tc.tile_pool(name="hq", bufs=2) as hqp,
            tc.tile_pool(name="outp", bufs=2) as outp,
            tc.tile_pool(name="pt", bufs=2, space="PSUM") as ptp,
            tc.tile_pool(name="pg", bufs=2, space="PSUM") as pgp,
            tc.tile_pool(name="pu", bufs=2, space="PSUM") as pup,
            tc.tile_pool(name="p2", bufs=2, space="PSUM") as p2p,
        ):
            env = dict(locals())
            ident = const.tile([128, 128], BF16)
            make_identity(nc, ident)
            env["ident"] = ident
            ident_f32 = const.tile([128, 128], F32)
            make_identity(nc, ident_f32)
            env["ident_f32"] = ident_f32
            ones_row = const.tile([128, 128], F32)
            nc.vector.memset(ones_row[:], 1.0)
            env["ones_row"] = ones_row
            for _rep in range(reps):
                if _rep > 0:
                    env["out_d"] = nc.dram_tensor(
                        f"out_rep{_rep}", [T_LOC, H], F32).ap()
                _emit_body(nc, tc, env)
    return nc


def _emit_body(nc, tc, pools):
    const = pools["const"]; xload = pools["xload"]; xqp = pools["xq"] if "xq" in pools else pools["xqp"]
    xqp = pools["xqp"]; xqtp = pools["xqtp"]; small = pools["small"]
    wload = pools["wload"]; wcast = pools["wcast"]; scalep = pools["scalep"]
    w2scalep = pools["w2scalep"]; hbuf = pools["hbuf"]; hqp = pools["hqp"]
    outp = pools["outp"]; ptp = pools["ptp"]; pgp = pools["pgp"]
    pup = pools["pup"]; p2p = pools["p2p"]
    x_d = pools["x_d"]; w13T_d = pools["w13T_d"]; w2T_d = pools["w2T_d"]
    xT_d = pools["xT_d"]
    wsg_d = pools["wsg_d"]; wsu_d = pools["wsu_d"]; w2s_d = pools["w2s_d"]
    out_d = pools["out_d"]
    ident = pools["ident"]
    ident_f32 = pools["ident_f32"]
    ones_row = pools["ones_row"]

    xqT = {}     # t-block -> [128h, KT1, 128t] bf16
    s1s = {}     # t-block -> [128, 1] f32 quant scale
    cast_n = [0]

    def cast(dst, src):
        _cast_engine(nc, cast_n[0])(dst, src)
        cast_n[0] += 1

    invbs = {}

    def quantize_scales(tb):
        # amax over the natural-layout row chunks (free-dim reduce)
        NQ = 8
        QW = H // NQ
        am = None
        for hh in range(NQ):
            xt = xload.tile([128, QW], F32, tag="xt", name=f"xt{tb}_{hh}",
                            bufs=2)
            nc.sync.dma_start(
                xt[:], x_d[tb * 128:(tb + 1) * 128, hh * QW:(hh + 1) * QW])
            amn = small.tile([128, 1], F32, tag="amax1", name=f"am{tb}_{hh}")
            nc.vector.tensor_reduce(amn[:], xt[:], axis=AX.X, op=ALU.max,
                                    apply_absolute_value=True)
            if hh > 0:
                am2 = small.tile([128, 1], F32, tag="amax1b",
                                 name=f"amc{tb}_{hh}")
                nc.vector.tensor_tensor(am2[:], am[:], amn[:], op=ALU.max)
                am = am2
            else:
                am = amn
        s1 = small.tile([128, 1], F32, tag="s1")
        nc.vector.tensor_scalar(s1[:], am[:], 1.0 / 127.0, None, op0=ALU.mult)
        inv1 = small.tile([128, 1], F32, tag="inv1")
        nc.vector.reciprocal(inv1[:], s1[:])
        s1s[tb] = s1
        return inv1

    def quantize_bounce(tb, inv1):
        # broadcast inv1 across partitions with a PE outer product
        pinv = ptp.tile([128, 128], F32, tag="pt", name="pinv")
        nc.tensor.transpose(pinv[0:1, :], inv1[:], ident_f32[:])
        invrow = small.tile([128, 128], F32, tag="invrow")
        nc.vector.tensor_copy(invrow[0:1, :], pinv[0:1, :])
        pbc = ptp.tile([128, 128], F32, tag="pt", name="pbc")
        nc.tensor.matmul(pbc[:], ones_row[0:1, :], invrow[0:1, :],
                         start=True, stop=True)
        invb = small.tile([128, 1, 128], F32, tag="invb")
        nc.vector.tensor_copy(invb[:, 0, :], pbc[:])
        invbs[tb] = invb

    def quantize_apply(tb):
        # quantize the host-pretransposed xT directly in [h, t] layout:
        # multiply by the broadcast scale, round to int8, cast to bf16.
        NQ = 4
        QW = H // NQ
        KQ = KT1 // NQ
        invb = invbs[tb]
        xqt = xqtp.tile([128, KT1, 128], BF16, tag="xqT")
        for hh in range(NQ):
            xTt = xload.tile([128, KQ, 128], F32, tag="xTt",
                             name=f"xTt{tb}_{hh}", bufs=2)
            nc.sync.dma_start(
                xTt[:],
                xT_d[hh * QW:(hh + 1) * QW, tb * 128:(tb + 1) * 128]
                .rearrange("(k p) t -> p k t", p=128))
            xq8 = xqp.tile([128, KQ, 128], I8, tag="xq_i8",
                           name=f"xq8_{hh}", bufs=2)
            nc.vector.scalar_tensor_tensor(
                xq8[:], xTt[:], 1.0, invb[:].broadcast_to([128, KQ, 128]),
                op0=ALU.mult, op1=ALU.mult)
            nc.scalar.copy(xqt[:, hh * KQ:(hh + 1) * KQ, :], xq8[:])
        xqT[tb] = xqt

    def mm1_loads(e, c0, cw):
        wg_i8 = [wload.tile([128, KT1 // 2, cw], I8, tag="wg_i8",
                            name=f"wg_i8_{e}_{c0}_{h2}") for h2 in range(2)]
        wu_i8 = [wload.tile([128, KT1 // 2, cw], I8, tag="wu_i8",
                            name=f"wu_i8_{e}_{c0}_{h2}") for h2 in range(2)]
        g_src = w13T_d[e, :, c0:c0 + cw].rearrange("(k p) o -> p k o", p=128)
        u_src = w13T_d[e, :, I + c0:I + c0 + cw].rearrange(
            "(k p) o -> p k o", p=128)
        for h2 in range(2):
            ksl = slice(h2 * (KT1 // 2), (h2 + 1) * (KT1 // 2))
            nc.sync.dma_start(wg_i8[h2][:], g_src[:, ksl, :])
            nc.sync.dma_start(wu_i8[h2][:], u_src[:, ksl, :])
        return wg_i8, wu_i8

    QK = 4  # k-tiles per cast op

    def cast_quad(w_i8, kq, cw, nm):
        h2, kkq = divmod(kq, (KT1 // 2) // QK)
        ks = slice(kkq * QK, (kkq + 1) * QK)
        w_bf = wcast.tile([128, QK, cw], BF16, tag="wbf", name=nm)
        cast(w_bf[:], w_i8[h2][:, ks, :])
        return w_bf

    def mm1_epilogue(e, i_tb, tb, c0, cw, pg, pu, wsg, wsu, hts, amaxes):
        gate = outp.tile([128, cw], F32, tag="gate")
        nc.vector.scalar_tensor_tensor(
            gate[:], pg, s1s[tb][:], wsg[:, c0:c0 + cw],
            op0=ALU.mult, op1=ALU.mult)
        up = outp.tile([128, cw], F32, tag="up")
        nc.vector.scalar_tensor_tensor(
            up[:], pu, s1s[tb][:], wsu[:, c0:c0 + cw],
            op0=ALU.mult, op1=ALU.mult)
        sg = outp.tile([128, cw], F32, tag="sg")
        nc.scalar.activation(sg[:], gate[:], AF.Silu)
        nc.vector.tensor_mul(hts[i_tb][:, c0:c0 + cw], sg[:], up[:])
        # per-chunk partial abs-max keeps the requant scale off the
        # critical path (ready right after the last chunk's h lands)
        prev = amaxes[i_tb]
        amp = small.tile([128, 1], F32, tag="amax2", name=f"am2p_{i_tb}_{c0}")
        nc.vector.tensor_reduce(amp[:], hts[i_tb][:, c0:c0 + cw],
                                axis=AX.X, op=ALU.max,
                                apply_absolute_value=True)
        if prev is not None:
            amn = small.tile([128, 1], F32, tag="amax2",
                             name=f"am2_{i_tb}_{c0}")
            nc.vector.tensor_tensor(amn[:], prev[:], amp[:], op=ALU.max)
            amaxes[i_tb] = amn
        else:
            amaxes[i_tb] = amp

    def mm1_chunk(e, tbs, c0, cw, wsg, wsu, hts, amaxes, loads,
                  precast=None):
        wg_i8, wu_i8 = loads
        pg = [pgp.tile([128, cw], F32, tag="pg", name=f"pg{i}")
              for i in range(len(tbs))]
        pu = [pup.tile([128, cw], F32, tag="pu", name=f"pu{i}")
              for i in range(len(tbs))]
        for kq in range(KT1 // QK):
            if precast is not None and kq < len(precast):
                wg_bf, wu_bf = precast[kq]
            else:
                wg_bf = cast_quad(wg_i8, kq, cw, "wg_bf")
                wu_bf = cast_quad(wu_i8, kq, cw, "wu_bf")
            for dk in range(QK):
                k = kq * QK + dk
                st, sp = (k == 0), (k == KT1 - 1)
                for i_tb, tb in enumerate(tbs):
                    nc.tensor.matmul(pg[i_tb][:], xqT[tb][:, k, :],
                                     wg_bf[:, dk, :], start=st, stop=sp)
                    nc.tensor.matmul(pu[i_tb][:], xqT[tb][:, k, :],
                                     wu_bf[:, dk, :], start=st, stop=sp)
        for i_tb, tb in enumerate(tbs):
            mm1_epilogue(e, i_tb, tb, c0, cw, pg[i_tb][:], pu[i_tb][:],
                         wsg, wsu, hts, amaxes)

    def mm1_chunk_split(e, tbs, c0, cw, wsg, wsu, hts, amaxes, loads, hqT,
                        s2s):
        # last chunk: separate per-tb passes (own casts) so tb0's requant
        # chain runs under tb1's matmuls instead of stalling the PE
        wg_i8, wu_i8 = loads
        for i_tb, tb in enumerate(tbs):
            pg = pgp.tile([128, cw], F32, tag="pg", name=f"pgs{i_tb}")
            pu = pup.tile([128, cw], F32, tag="pu", name=f"pus{i_tb}")
            for kq in range(KT1 // QK):
                wg_bf = cast_quad(wg_i8, kq, cw, "wg_bf")
                wu_bf = cast_quad(wu_i8, kq, cw, "wu_bf")
                for dk in range(QK):
                    k = kq * QK + dk
                    st, sp = (k == 0), (k == KT1 - 1)
                    nc.tensor.matmul(pg[:], xqT[tb][:, k, :],
                                     wg_bf[:, dk, :], start=st, stop=sp)
                    nc.tensor.matmul(pu[:], xqT[tb][:, k, :],
                                     wu_bf[:, dk, :], start=st, stop=sp)
            mm1_epilogue(e, i_tb, tb, c0, cw, pg[:], pu[:], wsg, wsu, hts,
                         amaxes)
            hqt, s2 = requant_tb(hts[i_tb], amaxes[i_tb])
            hqT.append(hqt)
            s2s.append(s2)

    def requant_dve(ht, amax2):
        s2 = small.tile([128, 1], F32, tag="s2")
        nc.vector.tensor_scalar(s2[:], amax2[:], 1.0 / 127.0, None,
                                op0=ALU.mult)
        inv2 = small.tile([128, 1], F32, tag="inv2")
        nc.vector.reciprocal(inv2[:], s2[:])
        hq_i8 = hqp.tile([128, I], I8, tag="hq_i8")
        nc.vector.tensor_scalar(hq_i8[:], ht[:], inv2[:], None, op0=ALU.mult)
        hq_bf = hqp.tile([128, I], BF16, tag="hq_bf")
        nc.scalar.copy(hq_bf[:], hq_i8[:])
        return hq_bf, s2

    def requant_pe(hq_bf):
        hqt = hqp.tile([128, KT2, 128], BF16, tag="hqT")
        for k0 in range(0, KT2, 8):
            kn = min(8, KT2 - k0)
            pt = ptp.tile([128, 8, 128], BF16, tag="pt", name="pt_hq")
            for dk in range(kn):
                k = k0 + dk
                nc.tensor.transpose(pt[:, dk, :],
                                    hq_bf[:, k * 128:(k + 1) * 128], ident[:])
            (nc.scalar.copy if (k0 // 8) % 2 else nc.vector.tensor_copy)(
                hqt[:, k0:k0 + kn, :], pt[:, 0:kn, :])
        return hqt

    def mm2_chunk(e, tbs, c0, cw, hqT, s2s, w2s):
        w2_i8 = wload.tile([128, KT2, cw], I8, tag="w2_i8")
        nc.sync.dma_start(
            w2_i8[:],
            w2T_d[e, :, c0:c0 + cw].rearrange("(k p) o -> p k o", p=128))
        p2 = [p2p.tile([128, cw], F32, tag="p2", name=f"p2_{i}")
              for i in range(len(tbs))]
        k = 0
        for qn in (4, 4, 3):
            w2_bf = wcast.tile([128, qn, cw], BF16, tag="wbf", name="w2_bf")
            cast(w2_bf[:], w2_i8[:, k:k + qn, :])
            for dk in range(qn):
                for i_tb in range(2):
                    nc.tensor.matmul(p2[i_tb][:], hqT[i_tb][:, k, :],
                                     w2_bf[:, dk, :], start=(k == 0),
                                     stop=(k == KT2 - 1))
                k += 1
        for i_tb, tb in enumerate(tbs):
            ot = outp.tile([128, cw], F32, tag="ot", bufs=4)
            nc.vector.scalar_tensor_tensor(
                ot[:], p2[i_tb][:], s2s[i_tb][:], w2s[:, c0:c0 + cw],
                op0=ALU.mult, op1=ALU.mult)
            nc.sync.dma_start(out_d[tb * 128:(tb + 1) * 128, c0:c0 + cw],
                              ot[:])

    # ---- Staged two-expert pipeline ----
    # Emission order == engine-queue order, so stages are interleaved to keep
    # the PE fed across quant/requant latency chains.
    assert E_LOC == 2
    loads_ = {}
    wsgs, wsus, htss, amaxs = {}, {}, {}, {}
    hqbfs, s2ss, hqTs = {}, {}, {}

    def mm1_front(e):
        wsg = scalep.tile([128, I], F32, tag="wsg", name=f"wsg{e}")
        nc.sync.dma_start(wsg[:], wsg_d[e])
        wsu = scalep.tile([128, I], F32, tag="wsu", name=f"wsu{e}")
        nc.sync.dma_start(wsu[:], wsu_d[e])
        wsgs[e], wsus[e] = wsg, wsu
        htss[e] = [hbuf.tile([128, I], F32, tag="ht", name=f"ht{e}_{i}")
                   for i in range(2)]
        amaxs[e] = [None, None]

    def mm1_run_chunk(e, ci, precast=None):
        tbs = [2 * e, 2 * e + 1]
        c0, cw = I_CHUNKS[ci]
        ld = loads_.pop((e, ci), None)
        if ld is None:
            ld = mm1_loads(e, c0, cw)
        mm1_chunk(e, tbs, c0, cw, wsgs[e], wsus[e], htss[e], amaxs[e], ld,
                  precast=precast)

    def req_dve(e):
        hqbfs[e], s2ss[e] = [], []
        for i_tb in range(2):
            hq_bf, s2 = requant_dve(htss[e][i_tb], amaxs[e][i_tb])
            hqbfs[e].append(hq_bf)
            s2ss[e].append(s2)

    def req_pe(e):
        hqTs[e] = [requant_pe(hqbfs[e][i_tb]) for i_tb in range(2)]

    # --- expert 0 front: tb0's amax stream first, then weights ---
    inv_a = quantize_scales(0)
    loads_[(0, 0)] = mm1_loads(0, *I_CHUNKS[0])
    precast0 = [(cast_quad(loads_[(0, 0)][0], kq, I_CHUNKS[0][1], "wg_bf"),
                 cast_quad(loads_[(0, 0)][1], kq, I_CHUNKS[0][1], "wu_bf"))
                for kq in range(3)]
    inv_b = quantize_scales(1)
    quantize_bounce(0, inv_a)
    quantize_apply(0)
    quantize_bounce(1, inv_b)
    quantize_apply(1)
    mm1_front(0)
    mm1_run_chunk(0, 0, precast=precast0)
    # expert-1 token quant hides under expert-0 mm1
    inv_c = quantize_scales(2)
    inv_d = quantize_scales(3)
    mm1_run_chunk(0, 1)
    quantize_bounce(2, inv_c)
    quantize_bounce(3, inv_d)
    quantize_apply(2)
    quantize_apply(3)
    mm1_front(1)
    mm1_run_chunk(0, 2)
    # requant scale chain (DVE/ACT only) for e0, then e1 matmuls keep PE busy
    req_dve(0)
    mm1_run_chunk(1, 0)
    # e0 hq transposes: hq_bf has long been ready by now
    req_pe(0)
    w2s0 = w2scalep.tile([128, H], F32, tag="w2s", name="w2s0")
    nc.sync.dma_start(w2s0[:], w2s_d[0])
    for ci in range(4):
        mm2_chunk(0, [0, 1], *H_CHUNKS[ci], hqTs[0], s2ss[0], w2s0)
    mm1_run_chunk(1, 1)
    mm1_run_chunk(1, 2)
    req_dve(1)
    # e0's mm2 tail covers e1's requant chain
    for ci in range(4, 8):
        mm2_chunk(0, [0, 1], *H_CHUNKS[ci], hqTs[0], s2ss[0], w2s0)
    req_pe(1)
    w2s1 = w2scalep.tile([128, H], F32, tag="w2s", name="w2s1")
    nc.sync.dma_start(w2s1[:], w2s_d[1])
    for ci in range(8):
        mm2_chunk(1, [2, 3], *H_CHUNKS[ci], hqTs[1], s2ss[1], w2s1)


_cached_nc = None


def _make_in_maps(x, w13, w2, w13_scale, smooth_scale_2, w2_scale):
    x = np.asarray(x, dtype=np.float32)
    w13 = np.asarray(w13).astype(np.int8, copy=False)
    w2 = np.asarray(w2).astype(np.int8, copy=False)
    w13_scale = np.asarray(w13_scale, dtype=np.float32)
    smooth_scale_2 = np.asarray(smooth_scale_2, dtype=np.float32)
    w2_scale = np.asarray(w2_scale, dtype=np.float32)

    # Fold the (linear) smooth scale into the up-projection dequant scale.
    wsu_full = w13_scale[:, I:] * smooth_scale_2          # [E, I]
    wsg_full = w13_scale[:, :I]                           # [E, I]

    in_maps = []
    for c in range(NCORES):
        es = slice(E_LOC * c, E_LOC * (c + 1))
        ts = slice(T_LOC * c, T_LOC * (c + 1))
        in_maps.append({
            "x": np.ascontiguousarray(x[ts]),
            "xT": np.ascontiguousarray(x[ts].T),
            "w13T": np.ascontiguousarray(w13[es].transpose(0, 2, 1)),
            "w2T": np.ascontiguousarray(w2[es].transpose(0, 2, 1)),
            "wsg": np.ascontiguousarray(
                np.broadcast_to(wsg_full[es, None, :], (E_LOC, 128, I))),
            "wsu": np.ascontiguousarray(
                np.broadcast_to(wsu_full[es, None, :], (E_LOC, 128, I))),
            "w2s": np.ascontiguousarray(
                np.broadcast_to(w2_scale[es, None, :], (E_LOC, 128, H))),
        })
    return in_maps


def _run(in_maps, **kwargs):
    global _cached_nc
    _install_compile_hook()
    if _cached_nc is None:
        _cached_nc = _build_program()
    return run_bass_kernel_spmd(_cached_nc, in_maps, list(range(NCORES)),
                                **kwargs)


def kernel(x, w13, w2, w13_scale, smooth_scale_2, w2_scale, expert_tokens):
    # expert_tokens describes the fixed equal contiguous grouping (the
    # reference ignores it); we rely on that same grouping.
    del expert_tokens
    in_maps = _make_in_maps(x, w13, w2, w13_scale, smooth_scale_2, w2_scale)
    res = _run(in_maps)
    return np.concatenate([res.results[c]["out"] for c in range(NCORES)],
                          axis=0)


def run_profiled(x, w13, w2, w13_scale, smooth_scale_2, w2_scale,
                 expert_tokens):
    """test.py helper: run with NTFF profiling, return BassKernelResults."""
    del expert_tokens
    in_maps = _make_in_maps(x, w13, w2, w13_scale, smooth_scale_2, w2_scale)
    return _run(in_maps, trace=True)
